# revision 8
# baseline (speedup 1.0000x reference)
"""GPTQ int4 dequant + matmul kernel for Trainium2, column-parallel over 8 cores.

Computes out = x @ dequant(qweight, qzeros, scales) + bias where
  qweight: [OC//8, IC_total] int32 (nibbles packed along OC rows)
  qzeros:  [G, IC_total//8]  int32 (nibbles packed along IC cols)
  scales:  [G, IC_total]     float32
  x:       [N, OC]           float32
  bias:    [IC_total]        float32
Sharding: IC (out_features) split across 8 cores; x replicated.

Per-core kernel structure (v2 — prologue overlapped with main loop):
  1. zp unpack via int16-lane shifts (4 instrs extract 2 nibbles each) +
     bf16 bias-bit trick (|0x4300 makes the int nibble bits the bf16 value
     128+v exactly) -> PE-transpose to [IC, G] so zp/s are per-partition
     scalars.
  2. qweight per j-tile (128 IC cols): DMA -> PE-transpose (bit-exact) ->
     int16-lane nibble unpack (strided writes) -> |0x4300 (packed int16,
     fast DVE mode) -> per-group tensor_scalar (exact (128+v)-(128+zp)
     subtract, then *s, one rounding to bf16) -> xbar transpose into the
     [OC-part, KT, chunk] weight tiles.
  3. Bias pre-broadcast to [128, IC] fp32 via a one-time K=1 fp32 matmul.
  4. Main loop as (token-tile, chunk) cells: 32 matmuls accumulate one
     psum bank, DVE drains psum + adds fp32 bias, ACT-queue DMAs out.
     Cells are scheduled so chunk-0 cells run while chunks 1-2 are still
     dequantizing (j-tile dequant interleaved into the cell stream);
     deferred cells revisit early token tiles at the end (x re-DMA'd).
  DMA queue split: gpsimd = x cast-loads + qweight loads; sync = consts +
     W xbar transposes; scalar(ACT) = xT transposes + output stores.
"""

import sys

if "/opt/trn_rl_repo" not in sys.path:
    sys.path.insert(0, "/opt/trn_rl_repo")

from contextlib import ExitStack

import numpy as np
import ml_dtypes

from concourse import bacc, bass, mybir, tile

P = 128
PACK = 8
BF16_BIAS = 0x4300  # bf16 bits of 128.0; |v -> bits of 128+v exactly (v in 0..15)

f32 = mybir.dt.float32
bf16 = mybir.dt.bfloat16
i32 = mybir.dt.int32
i16 = mybir.dt.int16
Alu = mybir.AluOpType

# Full problem dims (hardcoded per harness contract)
N_FULL = 4096
K_FULL = 4096  # OC / in_features (contraction)
IC_TOTAL = 11008
G_FULL = 32
N_CORES = 8
IC_SHARD = IC_TOTAL // N_CORES  # 1376


def _jtiles(ic):
    """IC j-tiles of <=128, last may be ragged (must stay %16 for xbar)."""
    tiles = []
    off = 0
    while off < ic:
        w = min(P, ic - off)
        assert w % 16 == 0, f"ragged j-tile {w} not multiple of 16"
        tiles.append((off, w))
        off += w
    return tiles


def _chunks(ic):
    """Greedy grouping of j-tiles into psum chunks of <=512 fp32."""
    chunks = []
    start = 0
    for off, w in _jtiles(ic):
        if off + w - start > 512:
            chunks.append((start, off - start))
            start = off
    chunks.append((start, ic - start))
    return chunks


def build(nc, n=N_FULL, k=K_FULL, ic=IC_SHARD, g=G_FULL, debug_dump=False):
    """Emit the per-core program. All cores run the same program (SPMD)."""
    assert k % P == 0 and n % P == 0 and k // g == P
    KT = k // P  # contraction tiles (each == one quant group)
    NT = n // P  # token tiles
    jts = _jtiles(ic)
    NJ = len(jts)
    chunks = _chunks(ic)
    NC = len(chunks)
    # map j-tile -> (chunk index, offset within chunk)
    jt_chunk = []
    for off, w in jts:
        for ci, (c0, cw) in enumerate(chunks):
            if c0 <= off < c0 + cw:
                jt_chunk.append((ci, off - c0))
                break
    # last j-tile of each chunk (chunk ready once that j-tile is transposed)
    chunk_last_jt = [max(ji for ji in range(NJ) if jt_chunk[ji][0] == ci)
                     for ci in range(NC)]

    q_d = nc.dram_tensor("qweight", [k // PACK, ic], i32, kind="ExternalInput")
    qz_d = nc.dram_tensor("qzeros", [g, ic // PACK], i32, kind="ExternalInput")
    s_d = nc.dram_tensor("scales", [g, ic], f32, kind="ExternalInput")
    x_d = nc.dram_tensor("x", [n, k], f32, kind="ExternalInput")
    b_d = nc.dram_tensor("bias", [ic], f32, kind="ExternalInput")
    id128_d = nc.dram_tensor("id128_f32", [P, P], f32, kind="ExternalInput")
    id32b_d = nc.dram_tensor("id32_bf16", [g, g], bf16, kind="ExternalInput")
    ones_d = nc.dram_tensor("ones_f32", [1, P], f32, kind="ExternalInput")
    out_d = nc.dram_tensor("out", [n, ic], f32, kind="ExternalOutput")
    if debug_dump:
        NJd = len(_jtiles(ic))
        dbg_zpT = nc.dram_tensor("dbg_zpT", [P, NJd, g], f32, kind="ExternalOutput")
        dbg_sT = nc.dram_tensor("dbg_sT", [P, NJd, g], f32, kind="ExternalOutput")
        dbg_bias = nc.dram_tensor("dbg_bias", [P, ic], f32, kind="ExternalOutput")
        dbg_xT = nc.dram_tensor("dbg_xT", [P, k // P, P], bf16, kind="ExternalOutput")
        dbg_ws = [nc.dram_tensor(f"dbg_w{ci}", [P, k // P, cw], bf16, kind="ExternalOutput")
                  for ci, (c0, cw) in enumerate(_chunks(ic))]

    with tile.TileContext(nc) as tc, ExitStack() as ctx:
        const = ctx.enter_context(tc.tile_pool(name="const", bufs=1))
        wpool = ctx.enter_context(tc.tile_pool(name="w", bufs=1))
        qw4p = ctx.enter_context(tc.tile_pool(name="qw4", bufs=2))
        qwTp = ctx.enter_context(tc.tile_pool(name="qwT", bufs=2))
        nibp = ctx.enter_context(tc.tile_pool(name="nib", bufs=1))
        wtp = ctx.enter_context(tc.tile_pool(name="wt", bufs=2))
        xbp = ctx.enter_context(tc.tile_pool(name="xb", bufs=2))
        xTp = ctx.enter_context(tc.tile_pool(name="xT", bufs=3))
        opool = ctx.enter_context(tc.tile_pool(name="o", bufs=4))
        psum = ctx.enter_context(tc.tile_pool(name="psum", bufs=4, space="PSUM"))
        psum_t = ctx.enter_context(tc.tile_pool(name="psum_t", bufs=2, space="PSUM"))

        # ---- constants (sync queue)
        id128 = const.tile([P, P], f32)
        nc.sync.dma_start(out=id128[:], in_=id128_d[:])
        id32b = const.tile([g, g], bf16)
        nc.sync.dma_start(out=id32b[:], in_=id32b_d[:])
        ones = const.tile([1, P], f32)
        nc.sync.dma_start(out=ones[:], in_=ones_d[:])
        bias_sb = const.tile([1, ic], f32)
        nc.sync.dma_start(out=bias_sb[:], in_=b_d[None, :])
        qz_sb = const.tile([g, ic // PACK], i32)
        nc.sync.dma_start(out=qz_sb[:], in_=qz_d[:])
        s_sb = const.tile([g, ic], f32)
        nc.sync.dma_start(out=s_sb[:], in_=s_d[:])

        # ---- bias broadcast to [128, ic] fp32 via K=1 fp32 matmul
        bias_bc = const.tile([P, ic], f32)
        for ci, (c0, cw) in enumerate(chunks):
            pb = psum.tile([P, 512], f32, name="ps")
            nc.tensor.matmul(
                pb[:, :cw], lhsT=ones[:, :], rhs=bias_sb[:, c0 : c0 + cw],
                start=True, stop=True,
            )
            nc.vector.tensor_copy(bias_bc[:, c0 : c0 + cw], pb[:, :cw])

        # ---- zp unpack: qzeros [g, ic//8] i32 -> zp16 [g, ic] i16
        #      (bits = bf16 of 128+zp).  int16 halves of each int32 hold
        #      nibbles 0-3 / 4-7; shift s extracts int16-elem j -> col 4j+s/4.
        zp16 = const.tile([g, ic], i16)
        qz16 = qz_sb.bitcast(i16)  # [g, ic//4]
        for s in (0, 4, 8, 12):
            nc.vector.tensor_scalar(
                out=zp16[:, s // 4 :: 4], in0=qz16[:], scalar1=s, scalar2=15,
                op0=Alu.logical_shift_right, op1=Alu.bitwise_and,
            )
        nc.vector.tensor_scalar(
            out=zp16[:], in0=zp16[:], scalar1=BF16_BIAS, scalar2=None,
            op0=Alu.bitwise_or,
        )

        # ---- transpose zp/s to [IC-part, NJ, g]
        zpT = const.tile([P, NJ, g], f32)  # value 128+zp (exact)
        sT = const.tile([P, NJ, g], f32)
        for ji, (off, w) in enumerate(jts):
            pz = psum_t.tile([P, g], bf16, name="pst")
            nc.tensor.transpose(
                pz[:w, :g], zp16.bitcast(bf16)[:, off : off + w], id32b[:]
            )
            nc.vector.tensor_copy(zpT[:w, ji, :], pz[:w, :g])
            ps_ = psum_t.tile([P, g], f32, name="pst")
            nc.tensor.transpose(ps_[:w, :g], s_sb[:, off : off + w], id128[:g, :g])
            nc.vector.tensor_copy(sT[:w, ji, :], ps_[:w, :g])

        # ---- W chunks in [OC-part, KT, chunk-width] bf16
        wtiles = [wpool.tile([P, KT, cw], bf16, name=f"Wc{ci}")
                  for ci, (c0, cw) in enumerate(chunks)]

        RP = k // PACK  # packed qweight rows (512)
        assert RP % P == 0
        NRT = RP // P

        def emit_jt(ji):
            """Dequantize j-tile ji into its W chunk slice."""
            off, w = jts[ji]
            qw4 = qw4p.tile([P, NRT, P], i32, name="qw4")
            for rt in range(NRT):
                r0 = rt * P
                nc.gpsimd.dma_start(
                    out=qw4[:, rt, :w], in_=q_d[r0 : r0 + P, off : off + w]
                )
            # PE-transpose (bit-exact) -> qwT [w, RP packed words]
            qwT = qwTp.tile([P, RP], i32, name="qwT")
            for rt in range(NRT):
                r0 = rt * P
                pq = psum_t.tile([P, P], f32, name="pst")
                nc.tensor.transpose(
                    pq[:w, :P], qw4.bitcast(f32)[:, rt, :w], id128[:]
                )
                nc.vector.tensor_copy(qwT.bitcast(f32)[:w, r0 : r0 + P], pq[:w, :P])
            # int16-lane nibble unpack: int16 elem j, shift s -> col 4j+s/4
            nib = nibp.tile([P, k], i16, name="nib")
            qw16 = qwT.bitcast(i16)  # [P, k//4]
            for s in (0, 4, 8, 12):
                nc.vector.tensor_scalar(
                    out=nib[:w, s // 4 :: 4], in0=qw16[:w, :], scalar1=s,
                    scalar2=15, op0=Alu.logical_shift_right, op1=Alu.bitwise_and,
                )
            nc.vector.tensor_scalar(
                out=nib[:w, :], in0=nib[:w, :], scalar1=BF16_BIAS, scalar2=None,
                op0=Alu.bitwise_or,
            )
            # dequant: WT = ((128+v) - (128+zp)) * s -> bf16 (exact sub, 1 round)
            wt = wtp.tile([P, k], bf16, name="wt")
            nibb = nib.bitcast(bf16)
            for gi in range(g):
                nc.vector.tensor_scalar(
                    out=wt[:w, gi * P : (gi + 1) * P],
                    in0=nibb[:w, gi * P : (gi + 1) * P],
                    scalar1=zpT[:w, ji, gi : gi + 1],
                    scalar2=sT[:w, ji, gi : gi + 1],
                    op0=Alu.subtract,
                    op1=Alu.mult,
                )
            ci, coff = jt_chunk[ji]
            nc.sync.dma_start_transpose(
                out=wtiles[ci][:, :, coff : coff + w], in_=wt[:w, :]
            )

        # ---- x tile prep: cast-load + xbar transpose (ACT queue)
        xcache = {}

        def emit_x(xkey):
            nt = xkey[0]
            xb = xbp.tile([P, k], bf16, name="xb")
            nc.gpsimd.dma_start(out=xb[:], in_=x_d[nt * P : (nt + 1) * P, :])
            xT = xTp.tile([P, KT, P], bf16, name="xT")
            nc.sync.dma_start_transpose(out=xT[:], in_=xb[:])
            xcache[xkey] = xT
            if debug_dump and xkey == (0, 0):
                nc.sync.dma_start(out=dbg_xT[:], in_=xT[:])

        def emit_cell(nt, ci, xkey):
            c0, cw = chunks[ci]
            xT = xcache[xkey]
            ps = psum.tile([P, 512], f32, name="ps")
            for kt in range(KT):
                nc.tensor.matmul(
                    ps[:, :cw],
                    lhsT=xT[:, kt, :],
                    rhs=wtiles[ci][:, kt, :],
                    start=(kt == 0),
                    stop=(kt == KT - 1),
                )
            osb = opool.tile([P, 512], f32, name="osb")
            nc.vector.tensor_tensor(
                out=osb[:, :cw], in0=ps[:, :cw], in1=bias_bc[:, c0 : c0 + cw],
                op=Alu.add,
            )
            nc.scalar.dma_start(
                out=out_d[nt * P : (nt + 1) * P, c0 : c0 + cw], in_=osb[:, :cw]
            )

        # ---- schedule: ramp cells on early chunks while later j-tiles
        #      dequantize; deferred cells revisit early nts at the end.
        sched = []
        RAMP = [(0, 0), ("J", 4), (1, 0), ("J", 5), (2, 0), ("J", 6), (3, 0),
                ("J", 7), (4, 0), ("J", 8), (5, 0), ("J", 9), (6, 0), ("J", 10),
                (6, 1), (7, 0), (7, 1), (8, 0), (8, 1), (9, 0), (9, 1)]
        sched += RAMP
        for nt in range(10, NT):
            for ci in range(NC):
                sched.append((nt, ci))
        for nt in range(6):          # revisit: remaining chunks of nts 0-5
            for ci in range(1, NC):
                sched.append((nt, ci, "r"))
        for nt in range(6, 10):      # revisit: last chunk of nts 6-9
            sched.append((nt, NC - 1, "r"))

        # sanity: every (nt, ci) exactly once
        seen = set()
        for it in sched:
            if it[0] == "J":
                continue
            nt, ci = it[0], it[1]
            assert (nt, ci) not in seen
            seen.add((nt, ci))
        assert len(seen) == NT * NC

        # x pass key per cell: first-pass cells of nt share (nt, 0);
        # revisit cells use (nt, 1) with a fresh load.
        xorder = []  # first-use order of xkeys, for prefetching
        for it in sched:
            if it[0] == "J":
                continue
            xk = (it[0], 1 if len(it) == 3 else 0)
            if xk not in xorder:
                xorder.append(xk)
        xpos = {xk: i for i, xk in enumerate(xorder)}

        # emit j-tiles 0..3 (chunk 0) up front
        for ji in range(4):
            emit_jt(ji)

        # emit cells + interleaved j-tiles, keeping x prefetch 2 keys ahead
        nxt = 0  # next xkey index to emit

        def prefetch(upto):
            nonlocal nxt
            while nxt < min(upto, len(xorder)):
                emit_x(xorder[nxt])
                nxt += 1

        prefetch(2)
        for it in sched:
            if it[0] == "J":
                emit_jt(it[1])
                continue
            nt, ci = it[0], it[1]
            xk = (nt, 1 if len(it) == 3 else 0)
            prefetch(xpos[xk] + 1)  # ensure this key is emitted
            emit_cell(nt, ci, xk)
            prefetch(xpos[xk] + 3)  # keep 2 keys ahead (xT ring has 3 bufs)

        if debug_dump:
            nc.sync.dma_start(out=dbg_zpT[:], in_=zpT[:])
            nc.sync.dma_start(out=dbg_sT[:], in_=sT[:])
            nc.sync.dma_start(out=dbg_bias[:], in_=bias_bc[:])
            for ci in range(NC):
                nc.sync.dma_start(out=dbg_ws[ci][:], in_=wtiles[ci][:])
    return nc


def make_const_inputs(g=G_FULL):
    return {
        "id128_f32": np.eye(P, dtype=np.float32),
        "id32_bf16": np.eye(g, dtype=ml_dtypes.bfloat16),
        "ones_f32": np.ones((1, P), dtype=np.float32),
    }


def kernel(input, qweight, qzeros, scales, bias):
    """Full-problem entry point: shard, run on 8 cores, gather."""
    from concourse.bass_utils import run_bass_kernel_spmd

    nc = bacc.Bacc("TRN2", target_bir_lowering=False, debug=False)
    build(nc)
    nc.compile()

    consts = make_const_inputs()
    x = np.ascontiguousarray(input, dtype=np.float32)
    in_maps = []
    for c in range(N_CORES):
        j0, j1 = c * IC_SHARD, (c + 1) * IC_SHARD
        in_maps.append(
            {
                "qweight": np.ascontiguousarray(qweight[:, j0:j1]),
                "qzeros": np.ascontiguousarray(
                    qzeros[:, c * (IC_SHARD // PACK) : (c + 1) * (IC_SHARD // PACK)]
                ),
                "scales": np.ascontiguousarray(scales[:, j0:j1]),
                "x": x,
                "bias": np.ascontiguousarray(bias[j0:j1]),
                **consts,
            }
        )
    res = run_bass_kernel_spmd(nc, in_maps, list(range(N_CORES)))
    outs = [np.asarray(res.results[c]["out"], dtype=np.float32) for c in range(N_CORES)]
    return np.concatenate(outs, axis=1)
